# revision 8
# baseline (speedup 1.0000x reference)
"""CAM (channel attention) module kernel for Trainium2, 8 NeuronCores.

Reference computation (per sample, x: [C, N] with C=512, N=64*64):
    energy    = x @ x.T                      # [C, C] Gram matrix
    att       = softmax(rowmax(energy) - energy, axis=-1)
    out       = gamma * (att @ x) + x

softmax(rowmax - e) == softmax(-e); stabilized with the row-min m_i:
att[i,j] = exp(m_i - e_ij) / S_i.

Sharding: pure data parallel over batch B=16 -> 2 samples per core.

v3 pipeline (attention branch fp8e4 + DoubleRow; epilogue exact f32,
emitted f16):
  1. load xf natural [4x128, N] f32 in 8 interleaved column pieces
     (sync queue)
  2. ACT casts nat -> m2m fp8 pair tiles [128, 2, N] (channel chunks
     2t/2t+1 side by side) - feed both PE transposes and mm2 moving
  3. PE-transpose fp8 chunks (psum step-2 writeback) -> xt pair tiles
     [128, 2, C]; psum->sbuf drain alternates DVE/ACT
  4. mm1 (triangular): symmetric energy, row panel ci computes columns
     [128*ci, 512) via fp8 DoubleRow (K=256/instr); missing lower
     blocks mirrored from stashed SBUF copies via f32 PE transposes
  5. softmax: DVE rowmin, ACT exp (psum read) with fused row-sum;
     paired block-diag D tiles [128, 2, 256] = diag(gamma/S) in fp8
  6. PT = P.T @ D as 8 DoubleRow matmuls (folds softmax norm + gamma)
  7. mm2: out_psum = sum_t ptp[t].T @ m2m[t] (DoubleRow); epilogue
     out = psum + x alternates DVE/gpsimd into [128, N] f16 row tiles;
     one merged store per channel block (gpsimd queue)

gamma=0 path is exact: D underflows to 0 in fp8 -> psum = 0 -> out =
f16(x) (~2^-11 max relative error from the f16 store). Engine load is
spread so the PE (the roofline engine) never waits on the psum drain:
ACT ~ casts+exp+half the xt drain, DVE ~ half xt drain + softmax small
ops + half epilogue, gpsimd ~ half epilogue + merged stores.
"""

import numpy as np

import concourse.bacc as bacc
import concourse.tile as tile
from concourse import mybir
from concourse.bass_utils import run_bass_kernel_spmd
from concourse.masks import make_identity

B, C, H, W = 16, 512, 64, 64
N = H * W
NCORES = 8
BPC = B // NCORES  # samples per core
CB = C // 128      # channel blocks (4)
NK = N // 128      # 128-wide n-chunks (32)
NP = NK // 2       # n-chunk pairs (16)
NT = N // 512      # 512-wide n-tiles (8)

F32 = mybir.dt.float32
F16 = mybir.dt.float16
FP8 = mybir.dt.float8e4
DR = mybir.MatmulPerfMode.DoubleRow
Copy = mybir.ActivationFunctionType.Copy


def _emit(nc, tc, ctx, x, gamma, out):
    consts = ctx.enter_context(tc.tile_pool(name="consts", bufs=1))
    nat_pool = ctx.enter_context(tc.tile_pool(name="nat", bufs=6))
    m2m_pool = ctx.enter_context(tc.tile_pool(name="m2m", bufs=4))
    xt_pool = ctx.enter_context(tc.tile_pool(name="xt", bufs=NP))
    pp_pool = ctx.enter_context(tc.tile_pool(name="pp", bufs=4))
    ptp_pool = ctx.enter_context(tc.tile_pool(name="ptp", bufs=4))
    dd_pool = ctx.enter_context(tc.tile_pool(name="dd", bufs=4))
    eblk_pool = ctx.enter_context(tc.tile_pool(name="eblk", bufs=8))
    small = ctx.enter_context(tc.tile_pool(name="small", bufs=4 * CB))
    outs_pool = ctx.enter_context(tc.tile_pool(name="outs", bufs=2))
    psum_e = ctx.enter_context(tc.tile_pool(name="psum_e", bufs=3, space="PSUM"))
    psum_g = ctx.enter_context(tc.tile_pool(name="psum_g", bufs=5, space="PSUM"))

    identity = consts.tile([128, 128], F32)
    make_identity(nc, identity[:])
    id8 = consts.tile([128, 128], FP8)
    nc.vector.tensor_copy(out=id8[:], in_=identity[:])
    wcon8 = consts.tile([128, 128], FP8)
    nc.vector.tensor_copy(out=wcon8[:], in_=identity[:])
    g_sb = consts.tile([128, 1], F32)
    nc.gpsimd.dma_start(out=g_sb[:], in_=gamma[:].to_broadcast((128, 1)))

    for s in range(BPC):
        # ---- load natural layout in 8 interleaved column pieces ----
        nat = [
            nat_pool.tile([128, N], F32, tag="nat", name=f"nat{s}_{c}")
            for c in range(CB)
        ]
        QN = N // 8
        for q in range(8):
            for c in range(CB):
                nc.sync.dma_start(
                    out=nat[c][:, QN * q : QN * (q + 1)],
                    in_=x[s, 128 * c : 128 * (c + 1), QN * q : QN * (q + 1)],
                )

        # ---- cast to fp8 pair tiles (ACT), piecewise behind the DMA ----
        m2m = [
            m2m_pool.tile([128, 2, N], FP8, tag="m2m", name=f"m2m{s}_{t}")
            for t in range(CB // 2)
        ]
        for q in range(8):
            for c in range(CB):
                nc.gpsimd.tensor_copy(
                    out=m2m[c // 2][:, c % 2, QN * q : QN * (q + 1)],
                    in_=nat[c][:, QN * q : QN * (q + 1)],
                )

        # keep the PE busy (HAM warm) while the first pieces land
        warm_ps = psum_g.tile([128, 128], F32, tag="g", name=f"warm{s}")
        nwarm = 16 if s == 0 else 8
        for w in range(nwarm):
            nc.tensor.matmul(warm_ps[:], wcon8[:], wcon8[:], start=(w == 0), stop=False)
        nc.tensor.matmul(warm_ps[:], wcon8[:], wcon8[:], start=False, stop=True)

        # ---- transposes (fp8, step-2 psum) + mm1 panel ci=0 ----
        # mm1 for pair qp-1 is emitted while pair qp transposes, so the
        # PE never waits on the drain it just enqueued.
        xts = []
        e_ps = [None] * CB
        e_ps[0] = psum_e.tile([128, C], F32, tag="e", name=f"e_ps{s}_0")

        def mm1_ci0(qp):
            nc.tensor.matmul(
                e_ps[0][:],
                xts[qp][:, :, 0:128],
                xts[qp][:, :, 0:C],
                start=(qp == 0),
                stop=(qp == NP - 1),
                perf_mode=DR,
            )

        for qp in range(NP):
            xt = xt_pool.tile([128, 2, C], FP8, tag="xt", name=f"xt{s}_{qp}")
            for half in range(2):
                k = 2 * qp + half
                t_ps = psum_g.tile([128, C, 2], FP8, tag="g", name=f"tps{s}_{k}")
                for c in range(CB):
                    nc.tensor.transpose(
                        t_ps[:, 128 * c : 128 * (c + 1), 0],
                        m2m[c // 2][:, c % 2, 128 * k : 128 * (k + 1)],
                        id8[:],
                    )
                nc.scalar.activation(
                    out=xt[:, half, :], in_=t_ps[:, :, 0],
                    func=Copy, bias=0.0, scale=1.0,
                )
            xts.append(xt)
            if qp >= 1:
                mm1_ci0(qp - 1)
        mm1_ci0(NP - 1)

        # ---- mm1 panels ci=1..3 (triangular) + mirror stash/restore ----
        # panel ci computes energy columns [128*ci, 512); block (ci, cj)
        # for cj > ci is stashed to SBUF, and panel cj later mirrors it
        # back (f32 PE transpose) into its missing column block.
        e_blk = {}
        for ci in range(1, CB):
            lo = 128 * ci
            e_ps[ci] = psum_e.tile([128, C], F32, tag="e", name=f"e_ps{s}_{ci}")
            for qp in range(NP):
                nc.tensor.matmul(
                    e_ps[ci][:, lo:C],
                    xts[qp][:, :, lo : lo + 128],
                    xts[qp][:, :, lo:C],
                    start=(qp == 0),
                    stop=(qp == NP - 1),
                    perf_mode=DR,
                )
        for ci in range(CB):
            lo = 128 * ci
            # stash blocks this panel provides to later panels
            for cj in range(ci + 1, CB):
                blk = eblk_pool.tile(
                    [128, 128], F32, tag="eblk", name=f"eblk{s}_{ci}_{cj}"
                )
                nc.vector.tensor_copy(
                    out=blk[:], in_=e_ps[ci][:, 128 * cj : 128 * (cj + 1)]
                )
                e_blk[(ci, cj)] = blk
            # mirror missing lower blocks from earlier panels
            for cj in range(ci):
                nc.tensor.matmul(
                    e_ps[ci][:, 128 * cj : 128 * (cj + 1)],
                    e_blk[(cj, ci)][:],
                    identity[:],
                    is_transpose=True,
                )

        # ---- softmax: P = exp(m - e) fp8, S = rowsum, gv = gamma/S ----
        pp = [
            pp_pool.tile([128, 2, C], FP8, tag="pp", name=f"pp{s}_{t}")
            for t in range(CB // 2)
        ]
        gvs = []
        for ci in range(CB):
            m = small.tile([128, 1], F32, tag="m")
            nc.vector.tensor_reduce(
                out=m[:], in_=e_ps[ci][:], axis=mybir.AxisListType.X,
                op=mybir.AluOpType.min,
            )
            ssum = small.tile([128, 1], F32, tag="s")
            nc.scalar.activation(
                out=pp[ci // 2][:, ci % 2, :],
                in_=e_ps[ci][:],
                func=mybir.ActivationFunctionType.Exp,
                bias=m[:], scale=-1.0, accum_out=ssum[:],
            )
            r = small.tile([128, 1], F32, tag="r")
            nc.vector.reciprocal(out=r[:], in_=ssum[:])
            gv = small.tile([128, 1], F32, tag="gv")
            nc.vector.tensor_mul(out=gv[:], in0=r[:], in1=g_sb[:])
            gvs.append(gv)

        # paired block-diag D: dd[t][:,0,0:128] = gv(2t) * I,
        #                      dd[t][:,1,128:256] = gv(2t+1) * I
        dd = []
        for t in range(CB // 2):
            d = dd_pool.tile([128, 2, 256], FP8, tag="dd", name=f"dd{s}_{t}")
            nc.gpsimd.memset(d[:], 0.0)
            nc.vector.tensor_scalar_mul(
                out=d[:, 0, 0:128], in0=identity[:], scalar1=gvs[2 * t][:]
            )
            nc.vector.tensor_scalar_mul(
                out=d[:, 1, 128:256], in0=identity[:], scalar1=gvs[2 * t + 1][:]
            )
            dd.append(d)

        # ---- PT = P.T @ D via DoubleRow: PT[j, i] = gamma * att[i, j] ----
        ptps = [
            psum_g.tile([128, C], F32, tag="g", name=f"ptp{s}_{bj}")
            for bj in range(CB)
        ]
        for t in range(CB // 2):
            for bj in range(CB):
                nc.tensor.matmul(
                    ptps[bj][:, 256 * t : 256 * (t + 1)],
                    pp[t][:, :, 128 * bj : 128 * (bj + 1)],
                    dd[t][:, :, 0:256],
                    start=True,
                    stop=True,
                    perf_mode=DR,
                )
        ptp = [
            ptp_pool.tile([128, 2, C], FP8, tag="pt", name=f"ptp8{s}_{t}")
            for t in range(CB // 2)
        ]
        for bj in range(CB):
            nc.scalar.activation(
                out=ptp[bj // 2][:, bj % 2, :], in_=ptps[bj][:],
                func=Copy, bias=0.0, scale=1.0,
            )

        # ---- out = PT.T @ m2m + x; merged f16 row stores ----
        for ci in range(CB):
            o_row = outs_pool.tile([128, N], F16, tag="o", name=f"orow{s}_{ci}")
            for nt in range(NT):
                ops = psum_g.tile([128, 512], F32, tag="g")
                for t in range(CB // 2):
                    nc.tensor.matmul(
                        ops[:],
                        ptp[t][:, :, 128 * ci : 128 * (ci + 1)],
                        m2m[t][:, :, 512 * nt : 512 * (nt + 1)],
                        start=(t == 0),
                        stop=(t == CB // 2 - 1),
                        perf_mode=DR,
                    )
                nc.vector.scalar_tensor_tensor(
                    out=o_row[:, 512 * nt : 512 * (nt + 1)],
                    in0=ops[:],
                    scalar=1.0,
                    in1=nat[ci][:, 512 * nt : 512 * (nt + 1)],
                    op0=mybir.AluOpType.bypass,
                    op1=mybir.AluOpType.add,
                )
            # split the merged stores over two queues
            eng = nc.gpsimd if ci % 2 == 0 else nc.sync
            eng.dma_start(
                out=out[s, 128 * ci : 128 * (ci + 1), :],
                in_=o_row[:],
            )


_NC_CACHE = None


def _build():
    global _NC_CACHE
    if _NC_CACHE is not None:
        return _NC_CACHE
    from contextlib import ExitStack

    nc = bacc.Bacc("TRN2", target_bir_lowering=False)
    x = nc.dram_tensor("x", [BPC, C, N], F32, kind="ExternalInput")
    gamma = nc.dram_tensor("gamma", [1, 1], F32, kind="ExternalInput")
    out = nc.dram_tensor("out", [BPC, C, N], F16, kind="ExternalOutput")
    with tile.TileContext(nc) as tc:
        with ExitStack() as ctx:
            _emit(nc, tc, ctx, x[:], gamma[:], out[:])
    nc.compile()
    _NC_CACHE = nc
    return nc


def kernel(x, gamma):
    x = np.ascontiguousarray(np.asarray(x, dtype=np.float32))
    gamma = np.ascontiguousarray(np.asarray(gamma, dtype=np.float32))
    assert x.shape == (B, C, H, W), x.shape
    xf = x.reshape(B, C, N)
    nc = _build()
    in_maps = [
        {
            "x": xf[c * BPC : (c + 1) * BPC],
            "gamma": gamma.reshape(1, 1),
        }
        for c in range(NCORES)
    ]
    res = run_bass_kernel_spmd(nc, in_maps, core_ids=list(range(NCORES)))
    out = np.concatenate(
        [np.asarray(res.results[c]["out"]) for c in range(NCORES)], axis=0
    )
    return out.astype(np.float32).reshape(B, C, H, W)


# revision 16
# speedup vs baseline: 1.3985x; 1.3985x over previous
"""CAM (channel attention) module kernel for Trainium2, 8 NeuronCores.

Reference computation (per sample, x: [C, N] with C=512, N=64*64):
    energy    = x @ x.T                      # [C, C] Gram matrix
    att       = softmax(rowmax(energy) - energy, axis=-1)
    out       = gamma * (att @ x) + x

softmax(rowmax - e) == softmax(-e); stabilized with the row-min m_i:
att[i,j] = exp(m_i - e_ij) / S_i.

Sharding: pure data parallel over batch B=16 -> 2 samples per core.

v3 pipeline (attention branch fp8e4 + DoubleRow; epilogue exact f32,
emitted f16):
  1. load xf natural [4x128, N] f32 in 8 interleaved column pieces
     (sync queue)
  2. ACT casts nat -> m2m fp8 pair tiles [128, 2, N] (channel chunks
     2t/2t+1 side by side) - feed both PE transposes and mm2 moving
  3. PE-transpose fp8 chunks (psum step-2 writeback) -> xt pair tiles
     [128, 2, C]; psum->sbuf drain alternates DVE/ACT
  4. mm1 (triangular): symmetric energy, row panel ci computes columns
     [128*ci, 512) via fp8 DoubleRow (K=256/instr); missing lower
     blocks mirrored from stashed SBUF copies via f32 PE transposes
  5. softmax: DVE rowmin, ACT exp (psum read) with fused row-sum;
     paired block-diag D tiles [128, 2, 256] = diag(gamma/S) in fp8
  6. PT = P.T @ D as 8 DoubleRow matmuls (folds softmax norm + gamma)
  7. mm2: out_psum = sum_t ptp[t].T @ m2m[t] (DoubleRow); epilogue
     out = psum + x alternates DVE/gpsimd into [128, N] f16 row tiles;
     one merged store per channel block (gpsimd queue)

gamma=0 path is exact: D underflows to 0 in fp8 -> psum = 0 -> out =
f16(x) (~2^-11 max relative error from the f16 store). Engine load is
spread so the PE (the roofline engine) never waits on the psum drain:
ACT ~ casts+exp+half the xt drain, DVE ~ half xt drain + softmax small
ops + half epilogue, gpsimd ~ half epilogue + merged stores.
"""

import numpy as np

import concourse.bacc as bacc
import concourse.tile as tile
from concourse import mybir
from concourse.bass_utils import run_bass_kernel_spmd
from concourse.masks import make_identity

B, C, H, W = 16, 512, 64, 64
N = H * W
NCORES = 8
BPC = B // NCORES  # samples per core
CB = C // 128      # channel blocks (4)
NK = N // 128      # 128-wide n-chunks (32)
NP = NK // 2       # n-chunk pairs (16)
NT = N // 512      # 512-wide n-tiles (8)

F32 = mybir.dt.float32
F16 = mybir.dt.float16
FP8 = mybir.dt.float8e4
DR = mybir.MatmulPerfMode.DoubleRow
Copy = mybir.ActivationFunctionType.Copy


def _emit(nc, tc, ctx, x, gamma, out):
    consts = ctx.enter_context(tc.tile_pool(name="consts", bufs=1))
    nat_pool = ctx.enter_context(tc.tile_pool(name="nat", bufs=6))
    m2m_pool = ctx.enter_context(tc.tile_pool(name="m2m", bufs=4))
    xt_pool = ctx.enter_context(tc.tile_pool(name="xt", bufs=NP))
    pp_pool = ctx.enter_context(tc.tile_pool(name="pp", bufs=4))
    ptp_pool = ctx.enter_context(tc.tile_pool(name="ptp", bufs=4))
    dd_pool = ctx.enter_context(tc.tile_pool(name="dd", bufs=2))
    eblk_pool = ctx.enter_context(tc.tile_pool(name="eblk", bufs=8))
    small = ctx.enter_context(tc.tile_pool(name="small", bufs=4 * CB))
    outs_pool = ctx.enter_context(tc.tile_pool(name="outs", bufs=2))
    psum_e = ctx.enter_context(tc.tile_pool(name="psum_e", bufs=3, space="PSUM"))
    psum_g = ctx.enter_context(tc.tile_pool(name="psum_g", bufs=5, space="PSUM"))

    identity = consts.tile([128, 128], F32)
    make_identity(nc, identity[:])
    id8 = consts.tile([128, 128], FP8)
    nc.vector.tensor_copy(out=id8[:], in_=identity[:])
    wcon8 = consts.tile([128, 128], FP8)
    nc.vector.tensor_copy(out=wcon8[:], in_=identity[:])
    g_sb = consts.tile([128, 1], F32)
    nc.gpsimd.dma_start(out=g_sb[:], in_=gamma[:].to_broadcast((128, 1)))

    # persistent paired block-diag D tiles: the zero quadrants are
    # memset once; only the gamma/S diagonal quadrants are rewritten
    # per sample ( dd[t][:,0,0:128] = gv(2t)*I, dd[t][:,1,128:256] =
    # gv(2t+1)*I )
    dd = [
        dd_pool.tile([128, 2, 256], FP8, tag="dd", name=f"dd_{t}")
        for t in range(CB // 2)
    ]
    for t in range(CB // 2):
        nc.gpsimd.memset(dd[t][:], 0.0)

    for s in range(BPC):
        # ---- load natural layout in 8 interleaved column pieces ----
        nat = [
            nat_pool.tile([128, N], F32, tag="nat", name=f"nat{s}_{c}")
            for c in range(CB)
        ]
        QN = N // 8
        for q in range(8):
            for c in range(CB):
                nc.sync.dma_start(
                    out=nat[c][:, QN * q : QN * (q + 1)],
                    in_=x[s, 128 * c : 128 * (c + 1), QN * q : QN * (q + 1)],
                )

        # ---- cast to fp8 pair tiles (ACT), piecewise behind the DMA ----
        m2m = [
            m2m_pool.tile([128, 2, N], FP8, tag="m2m", name=f"m2m{s}_{t}")
            for t in range(CB // 2)
        ]
        for q in range(8):
            for c in range(CB):
                # gpsimd is ~2.6x slower per element; give it 1 of 4
                if c == 3 and q % 2 == 0:
                    nc.gpsimd.tensor_copy(
                        out=m2m[c // 2][:, c % 2, QN * q : QN * (q + 1)],
                        in_=nat[c][:, QN * q : QN * (q + 1)],
                    )
                else:
                    nc.scalar.activation(
                        out=m2m[c // 2][:, c % 2, QN * q : QN * (q + 1)],
                        in_=nat[c][:, QN * q : QN * (q + 1)],
                        func=Copy, bias=0.0, scale=1.0,
                    )

        # keep the PE busy (HAM warm) while the first pieces land
        warm_ps = psum_g.tile([128, 128], F32, tag="g", name=f"warm{s}")
        nwarm = 16 if s == 0 else 8
        for w in range(nwarm):
            nc.tensor.matmul(warm_ps[:], wcon8[:], wcon8[:], start=(w == 0), stop=False)
        nc.tensor.matmul(warm_ps[:], wcon8[:], wcon8[:], start=False, stop=True)

        # ---- transposes (fp8, step-2 psum) + mm1 panel ci=0 ----
        # mm1 for pair qp-1 is emitted while pair qp transposes, so the
        # PE never waits on the drain it just enqueued.
        xts = []
        e_ps = [None] * CB
        e_ps[0] = psum_e.tile([128, C], F32, tag="e", name=f"e_ps{s}_0")

        def mm1_ci0(qp):
            nc.tensor.matmul(
                e_ps[0][:],
                xts[qp][:, :, 0:128],
                xts[qp][:, :, 0:C],
                start=(qp == 0),
                stop=(qp == NP - 1),
                perf_mode=DR,
            )

        for qp in range(NP):
            xt = xt_pool.tile([128, 2, C], FP8, tag="xt", name=f"xt{s}_{qp}")
            for half in range(2):
                k = 2 * qp + half
                t_ps = psum_g.tile([128, C, 2], FP8, tag="g", name=f"tps{s}_{k}")
                for c in range(CB):
                    nc.tensor.transpose(
                        t_ps[:, 128 * c : 128 * (c + 1), 0],
                        m2m[c // 2][:, c % 2, 128 * k : 128 * (k + 1)],
                        id8[:],
                    )
                if (2 * qp + half) % 8 < 3:
                    nc.vector.tensor_copy(out=xt[:, half, :], in_=t_ps[:, :, 0])
                else:
                    nc.scalar.activation(
                        out=xt[:, half, :], in_=t_ps[:, :, 0],
                        func=Copy, bias=0.0, scale=1.0,
                    )
            xts.append(xt)
            if qp >= 1:
                mm1_ci0(qp - 1)
        mm1_ci0(NP - 1)

        # ---- mm1 panels ci=1..3 (triangular) + mirror stash/restore ----
        # panel ci computes energy columns [128*ci, 512); block (ci, cj)
        # for cj > ci is stashed to SBUF, and panel cj later mirrors it
        # back (f32 PE transpose) into its missing column block.
        e_blk = {}
        for ci in range(1, CB):
            lo = 128 * ci
            e_ps[ci] = psum_e.tile([128, C], F32, tag="e", name=f"e_ps{s}_{ci}")
            for qp in range(NP):
                nc.tensor.matmul(
                    e_ps[ci][:, lo:C],
                    xts[qp][:, :, lo : lo + 128],
                    xts[qp][:, :, lo:C],
                    start=(qp == 0),
                    stop=(qp == NP - 1),
                    perf_mode=DR,
                )
        for ci in range(CB):
            lo = 128 * ci
            # stash blocks this panel provides to later panels
            for cj in range(ci + 1, CB):
                blk = eblk_pool.tile(
                    [128, 128], F32, tag="eblk", name=f"eblk{s}_{ci}_{cj}"
                )
                nc.vector.tensor_copy(
                    out=blk[:], in_=e_ps[ci][:, 128 * cj : 128 * (cj + 1)]
                )
                e_blk[(ci, cj)] = blk
            # mirror missing lower blocks from earlier panels
            for cj in range(ci):
                nc.tensor.matmul(
                    e_ps[ci][:, 128 * cj : 128 * (cj + 1)],
                    e_blk[(cj, ci)][:],
                    identity[:],
                    is_transpose=True,
                )

        # ---- softmax: P = exp(m - e) fp8, S = rowsum, gv = gamma/S ----
        pp = [
            pp_pool.tile([128, 2, C], FP8, tag="pp", name=f"pp{s}_{t}")
            for t in range(CB // 2)
        ]
        gvs = []
        for ci in range(CB):
            m = small.tile([128, 1], F32, tag="m")
            nc.vector.tensor_reduce(
                out=m[:], in_=e_ps[ci][:], axis=mybir.AxisListType.X,
                op=mybir.AluOpType.min,
            )
            ssum = small.tile([128, 1], F32, tag="s")
            nc.scalar.activation(
                out=pp[ci // 2][:, ci % 2, :],
                in_=e_ps[ci][:],
                func=mybir.ActivationFunctionType.Exp,
                bias=m[:], scale=-1.0, accum_out=ssum[:],
            )
            r = small.tile([128, 1], F32, tag="r")
            nc.vector.reciprocal(out=r[:], in_=ssum[:])
            gvs.append(r)

        # rewrite the diagonal quadrants: d = (I * (1/S)) * gamma in fp8
        for ci in range(CB):
            nc.gpsimd.tensor_scalar(
                out=dd[ci // 2][:, ci % 2, 128 * (ci % 2) : 128 * (ci % 2) + 128],
                in0=identity[:],
                scalar1=gvs[ci][:],
                scalar2=g_sb[:],
                op0=mybir.AluOpType.mult,
                op1=mybir.AluOpType.mult,
            )

        # ---- PT = P.T @ D via DoubleRow: PT[j, i] = gamma * att[i, j] ----
        ptps = [
            psum_g.tile([128, C], F32, tag="g", name=f"ptp{s}_{bj}")
            for bj in range(CB)
        ]
        for t in range(CB // 2):
            for bj in range(CB):
                nc.tensor.matmul(
                    ptps[bj][:, 256 * t : 256 * (t + 1)],
                    pp[t][:, :, 128 * bj : 128 * (bj + 1)],
                    dd[t][:, :, 0:256],
                    start=True,
                    stop=True,
                    perf_mode=DR,
                )
        ptp = [
            ptp_pool.tile([128, 2, C], FP8, tag="pt", name=f"ptp8{s}_{t}")
            for t in range(CB // 2)
        ]
        for bj in range(CB):
            if bj % 2 == 0:
                nc.vector.tensor_copy(out=ptp[bj // 2][:, bj % 2, :], in_=ptps[bj][:])
            else:
                nc.scalar.activation(
                    out=ptp[bj // 2][:, bj % 2, :], in_=ptps[bj][:],
                    func=Copy, bias=0.0, scale=1.0,
                )

        # ---- out = PT.T @ m2m + x; merged f16 row stores ----
        for ci in range(CB):
            o_row = outs_pool.tile([128, N], F16, tag="o", name=f"orow{s}_{ci}")
            for nt in range(NT):
                ops = psum_g.tile([128, 512], F32, tag="g")
                for t in range(CB // 2):
                    nc.tensor.matmul(
                        ops[:],
                        ptp[t][:, :, 128 * ci : 128 * (ci + 1)],
                        m2m[t][:, :, 512 * nt : 512 * (nt + 1)],
                        start=(t == 0),
                        stop=(t == CB // 2 - 1),
                        perf_mode=DR,
                    )
                nc.vector.scalar_tensor_tensor(
                    out=o_row[:, 512 * nt : 512 * (nt + 1)],
                    in0=ops[:],
                    scalar=1.0,
                    in1=nat[ci][:, 512 * nt : 512 * (nt + 1)],
                    op0=mybir.AluOpType.bypass,
                    op1=mybir.AluOpType.add,
                )
            nc.gpsimd.dma_start(
                out=out[s, 128 * ci : 128 * (ci + 1), :],
                in_=o_row[:],
            )


_NC_CACHE = None


def _build():
    global _NC_CACHE
    if _NC_CACHE is not None:
        return _NC_CACHE
    from contextlib import ExitStack

    nc = bacc.Bacc("TRN2", target_bir_lowering=False)
    x = nc.dram_tensor("x", [BPC, C, N], F32, kind="ExternalInput")
    gamma = nc.dram_tensor("gamma", [1, 1], F32, kind="ExternalInput")
    out = nc.dram_tensor("out", [BPC, C, N], F16, kind="ExternalOutput")
    with tile.TileContext(nc) as tc:
        with ExitStack() as ctx:
            _emit(nc, tc, ctx, x[:], gamma[:], out[:])
    nc.compile()
    _NC_CACHE = nc
    return nc


def kernel(x, gamma):
    x = np.ascontiguousarray(np.asarray(x, dtype=np.float32))
    gamma = np.ascontiguousarray(np.asarray(gamma, dtype=np.float32))
    assert x.shape == (B, C, H, W), x.shape
    xf = x.reshape(B, C, N)
    nc = _build()
    in_maps = [
        {
            "x": xf[c * BPC : (c + 1) * BPC],
            "gamma": gamma.reshape(1, 1),
        }
        for c in range(NCORES)
    ]
    res = run_bass_kernel_spmd(nc, in_maps, core_ids=list(range(NCORES)))
    out = np.concatenate(
        [np.asarray(res.results[c]["out"]) for c in range(NCORES)], axis=0
    )
    return out.astype(np.float32).reshape(B, C, H, W)
